# revision 2
# baseline (speedup 1.0000x reference)
"""AdaTT-with-shared-experts MoE forward on 8 Trainium2 NeuronCores.

Strategy (v2): pure data-parallel over batch. Each core gets B/8 = 128 rows
and all expert weights, pre-cast to fp16 host-side (fp16 matmuls run at the
same PE rate as bf16 but carry 10 mantissa bits, which keeps the fp16
vector-engine combine inside the error budget). All matmuls accumulate in
fp32 PSUM:

  - MM1 (x @ W1): stationary = W1 [d,h] chunks, moving = xT [d,b] -> hT psum
    in [h-partition, b-free] layout so MM2 needs no activation transpose.
    Contraction d-order is d = p*DC + dc to match the x-bar DMA transpose.
  - relu+bias on ScalarE/ACT (b1 fp32 per-partition bias), output fp16.
  - MM2 (h @ W2): stationary = hT chunks, moving = W2 [h,d] rows, N=512;
    b2 folded in as a K=1 rank-1 matmul against a ones row.
  - PSUM->SBUF evacuation (fp32 -> fp16) on the Pool engine.
  - Gating softmax on DVE/ACT (fp32); dense 18-expert combine via
    scalar_tensor_tensor on DVE with fp16 tensors (4x DVE mode) and fp32
    per-partition gate scalars.
  - Layer-0 -> layer-1 activation transpose via the x-bar DMA transpose
    (no PE transposes, no PSUM round-trip).

Self-contained: only needs numpy/jax/concourse (env-provided).
"""
import numpy as np
import ml_dtypes

import concourse.bass as bass
import concourse.tile as tile
from concourse import bacc, mybir

F16 = mybir.dt.float16
F32 = mybir.dt.float32
BF = mybir.dt.bfloat16
f16 = np.float16
bf16 = ml_dtypes.bfloat16

B, D, H = 1024, 512, 1024
T, NTE, NSE = 8, 2, 2
E = T * NTE + NSE          # 18
M0, M1 = T + 1, T          # modules with gates per layer
NCORES = 8
BC = B // NCORES           # 128 rows per core
DC, HC = D // 128, H // 128  # 4, 8 chunks

# ---------------------------------------------------------------- builder


def _emit_layer(tc, pools, io, lyr, m_out, src_xt, dst):
    """Emit one AdaTT layer for all 18 experts.

    src_xt: list of 9 SBUF tiles [128, DC, 128] f16 (xT per module; d index
            = p*DC + dc), or None for layer 0 (loaded from DRAM here).
    dst: ("xt", out_tiles_list) to produce next-layer xT tiles, or
         ("out", dram_ap) to write the final output.
    """
    nc = tc.nc
    (xt_pool, w_pool, ht_pool, eo_pool, acc_pool, g_pool, c_pool,
     ph_pool, peo_pool, pg_pool) = pools

    # ---- constants / small tensors for this layer
    b1sb = c_pool.tile([128, E, HC], F32, tag=f"b1_{lyr}")
    nc.gpsimd.dma_start(b1sb[:], io[f"b1h_{lyr}"][:])
    b2sb = c_pool.tile([1, E * 512], BF, tag=f"b2_{lyr}")
    nc.gpsimd.dma_start(b2sb[:], io[f"b2h_{lyr}"][:])
    bgsb = c_pool.tile([1, m_out * E], BF, tag=f"bg_{lyr}")
    nc.gpsimd.dma_start(bgsb[:], io[f"bgh_{lyr}"][:])
    addb = c_pool.tile([128, m_out * E], F32, tag=f"addb_{lyr}")
    nc.gpsimd.dma_start(addb[:], io[f"addb_{lyr}"][:])
    wgsb = c_pool.tile([128, m_out, DC, E], BF, tag=f"wg_{lyr}")
    nc.gpsimd.dma_start(wgsb[:], io[f"wgh_{lyr}"][:])
    ones = io["ones_sb"]

    # ---- xT tiles per module
    if src_xt is None:
        src_xt = []
        for m in range(M0):
            xt = xt_pool.tile([128, DC, 128], BF, tag=f"xt0_{m}")
            # ACT ring, not SWDGE: decouples this body's input load from the
            # previous body's out-write (both were FIFO on the SWDGE ring)
            nc.scalar.dma_start(xt[:], io["xt0"][m])
            src_xt.append(xt)

    # ---- expert weight prefetch (first expert before gates).
    # w1 and w2 are fused into one DRAM tensor so each expert is a single
    # large DMA (16KB/partition) — halves per-DMA queue overheads.
    def load_w(e):
        wt = w_pool.tile([128, DC * H + HC * 512], BF, tag="w")
        nc.sync.dma_start(wt[:], io[f"wh_{lyr}"][e])
        w1t = wt[:, :DC * H].rearrange("p (dc h) -> p dc h", h=H)
        w2t = wt[:, DC * H:].rearrange("p (hc d) -> p hc d", d=512)
        return w1t, w2t

    wq = [load_w(ep) for ep in range(3)]

    acc = [acc_pool.tile([128, 512], F16, tag=f"acc_{lyr}_{m}",
                         name=f"acc_{lyr}_{m}") for m in range(m_out)]
    if dst[0] == "xt":
        accb = [acc_pool.tile([128, 512], BF, tag=f"accb_{lyr}_{m}",
                              name=f"accb_{lyr}_{m}") for m in range(m_out)]
    else:
        accb = acc
    gf = g_pool.tile([128, m_out * E], F32, tag=f"gf_{lyr}")

    def emit_gates():
        # gate logits for all modules into one PSUM bank
        zg = pg_pool.tile([128, m_out * E], F32, tag="zg")
        for m in range(m_out):
            sl = zg[:, m * E:(m + 1) * E]
            nc.tensor.matmul(sl, ones[:], bgsb[:, m * E:(m + 1) * E],
                             start=True, stop=False)
            for dc in range(DC):
                nc.tensor.matmul(sl, src_xt[m][:, dc, :], wgsb[:, m, dc, :],
                                 start=False, stop=(dc == DC - 1))
        # softmax (+ self-exp residual add, pre-broadcast from host)
        zmaxn = g_pool.tile([128, m_out], F32, tag=f"zmx_{lyr}")
        nc.vector.tensor_reduce(zmaxn[:],
                                zg[:].rearrange("p (m e) -> p m e", e=E),
                                axis=mybir.AxisListType.X,
                                op=mybir.AluOpType.max, negate=True)
        ge = g_pool.tile([128, m_out * E], F32, tag=f"ge_{lyr}")
        for m in range(m_out):
            nc.scalar.activation(ge[:, m * E:(m + 1) * E],
                                 zg[:, m * E:(m + 1) * E],
                                 mybir.ActivationFunctionType.Exp,
                                 bias=zmaxn[:, m:m + 1], scale=1.0)
        esum = g_pool.tile([128, m_out], F32, tag=f"es_{lyr}")
        nc.vector.tensor_reduce(esum[:],
                                ge[:].rearrange("p (m e) -> p m e", e=E),
                                axis=mybir.AxisListType.X,
                                op=mybir.AluOpType.add)
        erec = g_pool.tile([128, m_out], F32, tag=f"er_{lyr}")
        nc.vector.reciprocal(erec[:], esum[:])
        for m in range(m_out):
            nc.vector.scalar_tensor_tensor(
                gf[:, m * E:(m + 1) * E], ge[:, m * E:(m + 1) * E],
                erec[:, m:m + 1], addb[:, m * E:(m + 1) * E],
                op0=mybir.AluOpType.mult, op1=mybir.AluOpType.add)

    # ---- experts
    for e in range(E):
        m = e // 2
        w1t, w2t = wq.pop(0)
        if e + 3 < E:
            wq.append(load_w(e + 3))

        # MM1: hT[hc] = sum_dc W1[dc,hc].T @ xT[dc]  -> [128h, 128b]
        ht = ht_pool.tile([128, HC, 128], BF, tag="ht")
        for hg in range(2):
            ph = ph_pool.tile([128, 4, 128], F32, tag="ph")
            for hq in range(4):
                hc = hg * 4 + hq
                for dc in range(DC):
                    nc.tensor.matmul(ph[:, hq, :],
                                     w1t[:, dc, hc * 128:(hc + 1) * 128],
                                     src_xt[m][:, dc, :],
                                     start=(dc == 0), stop=(dc == DC - 1))
            for hq in range(4):
                hc = hg * 4 + hq
                nc.scalar.activation(ht[:, hc, :], ph[:, hq, :],
                                     mybir.ActivationFunctionType.Relu,
                                     bias=b1sb[:, e, hc:hc + 1], scale=1.0)

        # gates fill the PE gap while expert 0's relu drains
        if e == 0:
            emit_gates()

        # MM2: eo = b2 + sum_hc hT[hc].T @ W2[hc]  -> [128b, 512d]
        peo = peo_pool.tile([128, 512], F32, tag="peo")
        nc.tensor.matmul(peo[:], ones[:], b2sb[:, e * 512:(e + 1) * 512],
                         start=True, stop=False)
        for hc in range(HC):
            nc.tensor.matmul(peo[:], ht[:, hc, :], w2t[:, hc, :],
                             start=False, stop=(hc == HC - 1))
        eo = eo_pool.tile([128, 512], F16, tag="eo")
        nc.scalar.copy(eo[:], peo[:])

        # combine into all module accumulators (fp16, fp32 gate scalars);
        # the final expert writes the bf16 copy used by the transpose.
        for m2 in range(m_out):
            gsl = gf[:, m2 * E + e: m2 * E + e + 1]
            dst_t = accb[m2] if e == E - 1 else acc[m2]
            if e == 0:
                nc.vector.tensor_scalar(dst_t[:], eo[:], gsl, None,
                                        op0=mybir.AluOpType.mult)
            else:
                nc.vector.scalar_tensor_tensor(dst_t[:], eo[:], gsl,
                                               acc[m2][:],
                                               op0=mybir.AluOpType.mult,
                                               op1=mybir.AluOpType.add)

    # ---- layer output
    kind, tgt = dst
    if kind == "xt":
        # NOTE: transposes ride the ACT DMA queue — the SP queue carries the
        # weight stream, and an in-order queue would stall layer-1 weight
        # prefetch behind these data-dependent transfers.
        for m in range(m_out):
            xt = xt_pool.tile([128, DC, 128], BF, tag=f"xt1_{m}")
            nc.scalar.dma_start_transpose(xt[:], accb[m][:])
            tgt.append(xt)
    else:
        for m in range(m_out):
            nc.gpsimd.dma_start(tgt[:, m, :], acc[m][:])


def _emit(tc, io):
    pools = tc._moe_pools
    c_pool = pools[6]
    nc = tc.nc
    ones_sb = c_pool.tile([1, 128], BF, tag="ones")
    nc.gpsimd.dma_start(ones_sb[:], io["ones"][:])
    io = dict(io)
    io["ones_sb"] = ones_sb
    xt1 = []
    _emit_layer(tc, pools, io, 0, M0, None, ("xt", xt1))
    _emit_layer(tc, pools, io, 1, M1, xt1, ("out", io["out"]))


def build(repeat=1):
    nc = bacc.Bacc("TRN2", target_bir_lowering=False, debug=False,
                   num_devices=NCORES)
    io = {}
    io["xt0"] = nc.dram_tensor("xt0", [M0, 128, DC, 128], BF,
                               kind="ExternalInput").ap()
    for lyr in range(2):
        m_out = M0 if lyr == 0 else M1
        io[f"wh_{lyr}"] = nc.dram_tensor(f"wh_{lyr}",
                                         [E, 128, DC * H + HC * 512], BF,
                                         kind="ExternalInput").ap()
        io[f"b1h_{lyr}"] = nc.dram_tensor(f"b1h_{lyr}", [128, E, HC], F32,
                                          kind="ExternalInput").ap()
        io[f"b2h_{lyr}"] = nc.dram_tensor(f"b2h_{lyr}", [1, E * 512], BF,
                                          kind="ExternalInput").ap()
        io[f"wgh_{lyr}"] = nc.dram_tensor(f"wgh_{lyr}", [128, m_out, DC, E],
                                          BF, kind="ExternalInput").ap()
        io[f"bgh_{lyr}"] = nc.dram_tensor(f"bgh_{lyr}", [1, m_out * E], BF,
                                          kind="ExternalInput").ap()
        io[f"addb_{lyr}"] = nc.dram_tensor(f"addb_{lyr}", [128, m_out * E],
                                           F32, kind="ExternalInput").ap()
    io["ones"] = nc.dram_tensor("ones", [1, 128], BF,
                                kind="ExternalInput").ap()
    io["out"] = nc.dram_tensor("out", [128, M1, 512], F16,
                               kind="ExternalOutput").ap()

    with tile.TileContext(nc) as tc:
        with (
            tc.tile_pool(name="xt", bufs=1) as xt_pool,
            tc.tile_pool(name="w", bufs=6) as w_pool,
            tc.tile_pool(name="ht", bufs=3) as ht_pool,
            tc.tile_pool(name="eo", bufs=3) as eo_pool,
            tc.tile_pool(name="acc", bufs=1) as acc_pool,
            tc.tile_pool(name="g", bufs=2) as g_pool,
            tc.tile_pool(name="const", bufs=1) as c_pool,
            tc.tile_pool(name="ph", bufs=4,
                         space=bass.MemorySpace.PSUM) as ph_pool,
            tc.tile_pool(name="peo", bufs=3,
                         space=bass.MemorySpace.PSUM) as peo_pool,
            tc.tile_pool(name="pg", bufs=1,
                         space=bass.MemorySpace.PSUM) as pg_pool,
        ):
            tc._moe_pools = (xt_pool, w_pool, ht_pool, eo_pool, acc_pool,
                             g_pool, c_pool, ph_pool, peo_pool, pg_pool)
            for _ in range(repeat):
                _emit(tc, io)
    nc.compile()
    return nc

# ---------------------------------------------------------------- host prep


def host_prep(inputs):
    """Build the 8 per-core input maps from the full-size problem inputs.

    Contraction-dim chunk order everywhere is d = dc*128 + p (p = partition,
    dc = chunk), matching the x-bar DMA transpose output layout
    (out[p, dc, b] = in[b, dc*128 + p]); hidden-dim order is h = hc*128 + p,
    matching MM1's PSUM column blocks.
    """
    x = np.asarray(inputs["inputs"], np.float32)
    common = {}
    for lyr in range(2):
        W1 = np.asarray(inputs[f"W1_{lyr}"], np.float32)
        W2 = np.asarray(inputs[f"W2_{lyr}"], np.float32)
        b1 = np.asarray(inputs[f"b1_{lyr}"], np.float32)
        b2 = np.asarray(inputs[f"b2_{lyr}"], np.float32)
        Wg = np.asarray(inputs[f"Wg_{lyr}"], np.float32)
        bg = np.asarray(inputs[f"bg_{lyr}"], np.float32)
        m_out = Wg.shape[0]
        w1h = np.ascontiguousarray(
            W1.reshape(E, DC, 128, H).transpose(0, 2, 1, 3)).astype(bf16)
        w2h = np.ascontiguousarray(
            W2.reshape(E, HC, 128, 512).transpose(0, 2, 1, 3)).astype(bf16)
        common[f"wh_{lyr}"] = np.concatenate(
            [w1h.reshape(E, 128, DC * H), w2h.reshape(E, 128, HC * 512)],
            axis=2)
        common[f"b1h_{lyr}"] = np.ascontiguousarray(
            b1.reshape(E, HC, 128).transpose(2, 0, 1)).astype(np.float32)
        common[f"b2h_{lyr}"] = b2.reshape(1, E * 512).astype(bf16)
        common[f"wgh_{lyr}"] = np.ascontiguousarray(
            Wg.reshape(m_out, DC, 128, E).transpose(2, 0, 1, 3)).astype(bf16)
        common[f"bgh_{lyr}"] = bg.reshape(1, m_out * E).astype(bf16)
    sew_task = np.asarray(inputs["sew_task"], np.float32)
    sew_shared = np.asarray(inputs["sew_shared"], np.float32)
    add0 = np.zeros((M0, E), np.float32)
    for m in range(T):
        add0[m, 2 * m:2 * m + 2] = sew_task[m, 0]
    add0[T, 2 * T:2 * T + 2] = sew_shared[0]
    add1 = np.zeros((M1, E), np.float32)
    for m in range(T):
        add1[m, 2 * m:2 * m + 2] = sew_task[m, 1]
    common["addb_0"] = np.broadcast_to(add0.reshape(1, -1),
                                       (128, M0 * E)).copy()
    common["addb_1"] = np.broadcast_to(add1.reshape(1, -1),
                                       (128, M1 * E)).copy()
    common["ones"] = np.ones((1, 128), bf16)

    in_maps = []
    for c in range(NCORES):
        xs = x[c * BC:(c + 1) * BC]                       # [128, 9, 512]
        xt0 = np.ascontiguousarray(
            xs.reshape(BC, M0, DC, 128).transpose(1, 3, 2, 0)).astype(bf16)
        m = dict(common)
        m["xt0"] = xt0
        in_maps.append(m)
    return in_maps

# ---------------------------------------------------------------- run


_CACHE = {}


def _runner(nc):
    """jit-once runner over 8 axon devices (shard_map over core axis)."""
    import jax
    from jax.sharding import Mesh, PartitionSpec
    from jax.experimental.shard_map import shard_map
    from concourse.bass2jax import (_bass_exec_p, install_neuronx_cc_hook,
                                    partition_id_tensor)

    install_neuronx_cc_hook()
    partition_name = (nc.partition_id_tensor.name
                      if nc.partition_id_tensor else None)
    in_names, out_names, out_avals = [], [], []
    for alloc in nc.m.functions[0].allocations:
        if not isinstance(alloc, mybir.MemoryLocationSet):
            continue
        name = alloc.memorylocations[0].name
        if alloc.kind == "ExternalInput":
            if name != partition_name:
                in_names.append(name)
        elif alloc.kind == "ExternalOutput":
            out_names.append(name)
            out_avals.append(jax.core.ShapedArray(
                tuple(alloc.tensor_shape), mybir.dt.np(alloc.dtype)))
    all_in = list(in_names) + list(out_names)
    if partition_name is not None:
        all_in.append(partition_name)

    def _body(*args):
        operands = list(args)
        if partition_name is not None:
            operands.append(partition_id_tensor())
        return tuple(_bass_exec_p.bind(
            *operands,
            out_avals=tuple(out_avals), in_names=tuple(all_in),
            out_names=tuple(out_names), lowering_input_output_aliases=(),
            sim_require_finite=True, sim_require_nnan=True, nc=nc))

    devices = jax.devices()[:NCORES]
    mesh = Mesh(np.asarray(devices), ("core",))
    nin = len(in_names) + len(out_names)
    fn = jax.jit(shard_map(_body, mesh=mesh,
                           in_specs=(PartitionSpec("core"),) * nin,
                           out_specs=(PartitionSpec("core"),) * len(out_names),
                           check_rep=False), keep_unused=True)
    return fn, in_names, out_names, out_avals


def get_exec(repeat=1):
    """Returns (fn, make_args, out_names, out_avals); caches across calls."""
    import jax
    key = ("exec", repeat)
    if key in _CACHE:
        return _CACHE[key]
    nc = build(repeat=repeat)
    fn, in_names, out_names, out_avals = _runner(nc)

    def make_args(in_maps):
        from jax.sharding import Mesh, NamedSharding, PartitionSpec
        mesh = Mesh(np.asarray(jax.devices()[:NCORES]), ("core",))
        sh = NamedSharding(mesh, PartitionSpec("core"))
        cat = [np.concatenate([m[k] for m in in_maps], axis=0)
               for k in in_names]
        zeros = [np.zeros((NCORES * a.shape[0],) + a.shape[1:], a.dtype)
                 for a in out_avals]
        return [jax.device_put(a, sh) for a in cat + zeros]

    _CACHE[key] = (fn, make_args, out_names, out_avals)
    return _CACHE[key]


def kernel(**inputs):
    import jax
    fn, make_args, out_names, out_avals = get_exec(repeat=1)
    in_maps = host_prep(inputs)
    last_err = None
    for attempt in range(3):
        try:
            args = make_args(in_maps)
            outs = jax.block_until_ready(fn(*args))
            out = np.asarray(outs[0]).astype(np.float32)
            return out.reshape(B, M1, 512)
        except Exception as e:                # transient runtime desync
            last_err = e
            time_mod = __import__("time")
            time_mod.sleep(2.0)
    raise last_err

